# revision 7
# baseline (speedup 1.0000x reference)
"""AdaptiveLowPassFilter Trainium2 kernel v4 — 8 NeuronCores, batch-parallel.

Measured-on-HW design points (vs v3's 281us / v2's 138us):
  - Products use tensor_tensor (2x_1p = 958ns per [128,1536] band-tap on
    HW). scalar_tensor_tensor never exceeds 1x on real TRN2 despite the
    cost model's 4x_2p table.
  - Intra-dj-column adds ride SWDGE accumulating DMAs (accum_op=add):
    ~0.94us of Pool-engine trigger each, transfer on idle DMA engines —
    the DVE keeps only the 72 products.
  - Phase A: fp8e4 DoubleRow pairs (2 taps/stream, j-stride must be
    EVEN on HW -> taps paired by delta parity). Prelu evac with an AP
    alpha (Lrelu's immediate alpha is silently ignored by HW) and
    scale=2^-8 compensating the fp8 weight pre-scale; 2-chunk paired
    evacs ([48,1024] across 2 psum banks) halve ACT instruction count.
  - Phase C dj-shift via shifted-identity matmuls (eye(k=+-1)) with
    kwA/kwB partition-shifted kw copies (HWDGE queue; SWDGE wedges the
    device on 3D strided copies).
"""
import sys
sys.path.insert(0, "/opt/trn_rl_repo")

import numpy as np
import ml_dtypes
from contextlib import ExitStack

C, CO, H, W, K = 96, 48, 128, 128, 9
RS = 130            # padded row stride in flat pixel space (and h stride)
PIX0 = 131          # flat offset of pixel (0, 0)
XBF = 17160         # x_flat free size (132 rows x 130)
NQ = 16896          # 33 chunks x 512 of h2 pixel space
NCHUNK = 33
CHUNK = 512
HB = 16             # band height (rows)
NB = H // HB        # 8 bands
WS = 8              # phase A weight pre-scale exponent (2^WS)

# tap deltas in flat pixel space, k = 3*di + dj
DELTA = [(di - 1) * RS + (dj - 1) for di in range(3) for dj in range(3)]
# DoubleRow tap pairs: j-stride (delta difference) must be even on HW
PAIRS = [(0, 2), (3, 5), (6, 8), (1, 4), (7, None)]
DUMMY_DELTA = 132   # valid x_flat memory, zero weights; 132-130=2 even

_CACHE = {}


def _build():
    import os
    import concourse.bass as bass
    import concourse.bacc as bacc
    import concourse.tile as tile
    import concourse.mybir as mybir

    DVE_ADDS = os.environ.get("V4_SWDGE_ADDS") != "1"
    POOL_TAPS = tuple(
        int(t) for t in os.environ.get("V4_POOL_TAPS", "8").split(",") if t)
    PE_BANDS = tuple(
        int(b) for b in os.environ.get("V4_PE_BANDS", "2,5").split(",") if b)
    POOL_REDUCE = os.environ.get("V4_POOL_REDUCE", "0") == "1"

    dt = mybir.dt
    f32, bf16, fp8 = dt.float32, dt.bfloat16, dt.float8e4
    AF = mybir.ActivationFunctionType
    OP = mybir.AluOpType
    PM = mybir.MatmulPerfMode

    nc = bacc.Bacc("TRN2", target_bir_lowering=False, debug=False)
    xf_d = nc.dram_tensor("x_flat", (C, XBF), fp8, kind="ExternalInput")
    xt_d = nc.dram_tensor("xt2c", (W, C * RS), bf16, kind="ExternalInput")
    wk5_d = nc.dram_tensor("wk5", (C, 5 * 2 * CO), fp8, kind="ExternalInput")
    pw2t_d = nc.dram_tensor("pw2t", (CO + 1, K), bf16, kind="ExternalInput")
    bh2_d = nc.dram_tensor("bh2", (CO, 1), f32, kind="ExternalInput")
    al_d = nc.dram_tensor("al", (CO, 1), f32, kind="ExternalInput")
    smat_d = nc.dram_tensor("smat", (W, 3 * W), bf16, kind="ExternalInput")
    ones_d = nc.dram_tensor("ones", (1, NQ), bf16, kind="ExternalInput")
    y_d = nc.dram_tensor("y", (W, NB * C * HB), bf16, kind="ExternalOutput")

    with ExitStack() as ctx:
        tc = ctx.enter_context(tile.TileContext(nc))
        st = ctx.enter_context(tc.tile_pool(name="st", bufs=1))
        prp = ctx.enter_context(tc.tile_pool(name="prp", bufs=12))
        pdp = ctx.enter_context(tc.tile_pool(name="pdp", bufs=6))
        nump = ctx.enter_context(tc.tile_pool(name="nump", bufs=3))
        h2p = ctx.enter_context(tc.tile_pool(name="h2p", bufs=2, space="PSUM"))
        ltp = ctx.enter_context(tc.tile_pool(name="ltp", bufs=2, space="PSUM"))
        pcp = ctx.enter_context(tc.tile_pool(name="pcp", bufs=2, space="PSUM"))

        x_flat = st.tile([C, XBF], fp8, tag="x_flat")
        xt2c = st.tile([W, C * RS], bf16, tag="xt2c")
        h2a = st.tile([W, NQ], bf16, tag="h2a")
        e_t = st.tile([W, K * H], bf16, tag="e_t")
        kw2 = st.tile([W, K * H], bf16, tag="kw2")
        kwA = st.tile([W, K * H], bf16, tag="kwA")
        kwB = st.tile([W, K * H], bf16, tag="kwB")
        den = st.tile([W, H], f32, tag="den")
        recip = st.tile([W, H], f32, tag="recip")
        wk5_sb = st.tile([C, 5 * 2 * CO], fp8, tag="wk5_sb")
        pw2t_sb = st.tile([CO + 1, K], bf16, tag="pw2t_sb")
        bh2_sb = st.tile([CO, 1], f32, tag="bh2_sb")
        al_sb = st.tile([CO, 1], f32, tag="al_sb")
        smat_sb = st.tile([W, 3 * W], bf16, tag="smat_sb")

        # ---- params (small, first; smat leads so PE warm-up starts early)
        nc.scalar.dma_start(smat_sb[:], smat_d.ap())
        nc.scalar.dma_start(wk5_sb[:], wk5_d.ap())
        nc.scalar.dma_start(pw2t_sb[:], pw2t_d.ap())
        nc.scalar.dma_start(bh2_sb[:], bh2_d.ap())
        nc.scalar.dma_start(al_sb[:], al_d.ap())
        nc.scalar.dma_start(h2a[CO:CO + 1, :], ones_d.ap())

        # ---- bulk inputs; x_flat leads on the SP queue, xt2c h-slabs on
        # the scalar queue (c-major layout -> strided pieces).
        XCUT = [0, 2145, 4290, 8580, 12870, XBF]
        for j in range(2):
            nc.sync.dma_start(
                x_flat[:, XCUT[j]:XCUT[j + 1]],
                xf_d.ap()[:, XCUT[j]:XCUT[j + 1]])
        HCUT = [0, 34, 66, 98, RS]
        xt_v = xt2c[:].rearrange("p (c h) -> p c h", h=RS)
        xtd_v = xt_d.ap().rearrange("p (c h) -> p c h", h=RS)
        nc.scalar.dma_start(xt_v[:, :, HCUT[0]:HCUT[1]],
                            xtd_v[:, :, HCUT[0]:HCUT[1]])
        for j in range(2, 5):
            nc.sync.dma_start(
                x_flat[:, XCUT[j]:XCUT[j + 1]],
                xf_d.ap()[:, XCUT[j]:XCUT[j + 1]])
        for j in range(1, 4):
            nc.scalar.dma_start(xt_v[:, :, HCUT[j]:HCUT[j + 1]],
                                xtd_v[:, :, HCUT[j]:HCUT[j + 1]])

        # ---- PE warm-up: real matmuls so HAM reaches 2.4 GHz
        wup = pcp.tile([W, CHUNK], f32, tag="pc_ps")
        for _ in range(12):
            nc.tensor.matmul(wup[:, 0:W], lhsT=smat_sb[:, W:2 * W],
                             rhs=smat_sb[:, W:2 * W], start=True, stop=True)

        # ---- phase A: fused dw*pw1 conv, fp8 DoubleRow tap pairs;
        # chunks processed in PAIRS sharing a 2-bank psum tile so one
        # Prelu evac covers 1024 columns.
        lhs5 = wk5_sb[:].rearrange("c (p j o) -> c p j o", p=5, j=2)
        xf_ap = x_flat[:]

        def emit_chunk_pair(i0, n):
            ps = h2p.tile([CO, 2 * CHUNK], f32, tag="h2ps")
            for t in range(n):
                q0 = PIX0 + CHUNK * (i0 + t)
                blk = ps[:, t * CHUNK:(t + 1) * CHUNK]
                for p, (ka, kb) in enumerate(PAIRS):
                    da = DELTA[ka]
                    db = DELTA[kb] if kb is not None else DUMMY_DELTA
                    rhs = bass.AP(xf_ap.tensor, xf_ap.offset + q0 + da,
                                  [[XBF, C], [db - da, 2], [1, CHUNK]])
                    nc.tensor.matmul(blk, lhsT=lhs5[:, p], rhs=rhs,
                                     start=(p == 0), stop=(p == 4),
                                     perf_mode=PM.DoubleRow)
            nc.scalar.activation(
                h2a[0:CO, CHUNK * i0:CHUNK * (i0 + n)], ps[:, 0:n * CHUNK],
                AF.Prelu, bias=bh2_sb[:], scale=2.0 ** -WS, alpha=al_sb[:],
            )

        # ---- phase B + softmax weights for a run of rows; e_t/kw2 are
        # stored [w, (k, h)] (h contiguous). kwA/kwB are the +/-1-partition
        # shifted copies of kw2.
        ev = e_t[:].rearrange("p (k h) -> p k h", h=H)
        kv2 = kw2[:].rearrange("p (k h) -> p k h", h=H)
        kvA = kwA[:].rearrange("p (k h) -> p k h", h=H)
        kvB = kwB[:].rearrange("p (k h) -> p k h", h=H)

        def emit_b(r0, rn):
            lt = ltp.tile([W, 2 * HB * K], f32, tag="lt")
            for r in range(rn):
                h = r0 + r
                nc.tensor.matmul(
                    lt[:, r * K:(r + 1) * K],
                    lhsT=h2a[0:CO + 1, h * RS:h * RS + W],
                    rhs=pw2t_sb[:], start=True, stop=True)
            ltv = lt[:, 0:rn * K].rearrange("p (h k) -> p k h", k=K)
            eb = ev[:, :, r0:r0 + rn]
            nc.scalar.activation(eb, ltv, AF.Exp)
            db = den[:, r0:r0 + rn]
            red_eng = nc.gpsimd if POOL_REDUCE else nc.vector
            red_eng.tensor_reduce(
                db, eb.transpose([0, 2, 1]),
                axis=mybir.AxisListType.X, op=OP.add)
            rb = recip[:, r0:r0 + rn]
            nc.vector.reciprocal(rb, db)
            nc.vector.tensor_tensor(
                kv2[:, :, r0:r0 + rn], eb,
                rb.unsqueeze(1).broadcast_to([W, K, rn]),
                op=OP.mult,
            )
            # kwA[v] = kw2[v+1] (needs dj=0 taps k=0,3,6): lhsT=eye(k=-1);
            # kwB[v] = kw2[v-1] (dj=2 taps k=2,5,8): lhsT=eye(k=+1).
            # psum boundary rows come out zero, exactly what the conv pads
            # need.
            for tgt, blk, k0 in ((kvA, 2, 0), (kvB, 0, 2)):
                kps = pcp.tile([W, CHUNK], f32, tag="pc_ps")
                rhs = bass.AP(kv2.tensor, kv2.offset + k0 * H + r0,
                              [[K * H, W], [3 * H, 3], [1, rn]])
                nc.tensor.matmul(kps[:, 0:3 * rn],
                                 lhsT=smat_sb[:, blk * W:(blk + 1) * W],
                                 rhs=rhs, start=True, stop=True)
                out = bass.AP(tgt.tensor, tgt.offset + k0 * H + r0,
                              [[K * H, W], [3 * H, 3], [1, rn]])
                nc.scalar.copy(out, kps[:, 0:3 * rn])

        KWT = {0: kvA, 1: kv2, 2: kvB}
        xcv = xt2c[:].rearrange("p (c h) -> p c h", h=RS)

        def xview(r0, di):
            return xcv[:, :, r0 + di:r0 + di + HB]

        def kwview(r0, k):
            return (KWT[k % 3][:, k, r0:r0 + HB]
                    .unsqueeze(1).broadcast_to([W, C, HB]))

        # ---- phase C: per-pixel 3x3 apply. Products on DVE (+ a few on
        # Pool); the 2 intra-dj adds per column ride SWDGE accum-DMAs; the
        # dj w-shift via shifted-identity matmuls on PE; evac on ACT.
        pd_of = {}

        def emit_c_products(b):
            r0 = b * HB
            # pool-tap products first so the Pool engine leads the chain
            pool_prods = {}
            for di in range(3):
                for dj in range(3):
                    k = 3 * di + dj
                    if k in POOL_TAPS:
                        prod = prp.tile([W, C * HB], bf16, tag="prod")
                        pv = prod[:].rearrange("p (c h) -> p c h", h=HB)
                        nc.gpsimd.tensor_tensor(
                            pv, xview(r0, di), kwview(r0, k), op=OP.mult)
                        pool_prods[k] = prod
            if b in PE_BANDS:
                prods = []
                for di in range(3):
                    for dj in range(3):
                        k = 3 * di + dj
                        if k in pool_prods:
                            prods.append((dj, pool_prods[k]))
                            continue
                        prod = prp.tile([W, C * HB], bf16, tag="prod")
                        pv = prod[:].rearrange("p (c h) -> p c h", h=HB)
                        nc.vector.tensor_tensor(
                            pv, xview(r0, di), kwview(r0, k), op=OP.mult)
                        prods.append((dj, prod))
                pd_of[b] = ("pe", prods)
                return
            pds = []
            for dj in range(3):
                pd = pdp.tile([W, C * HB], bf16, tag="pd")
                pdv = pd[:].rearrange("p (c h) -> p c h", h=HB)
                nc.vector.tensor_tensor(pdv, xview(r0, 0), kwview(r0, dj),
                                        op=OP.mult)
                for di in (1, 2):
                    k = 3 * di + dj
                    if k in pool_prods:
                        prod = pool_prods[k]
                    else:
                        prod = prp.tile([W, C * HB], bf16, tag="prod")
                        pv = prod[:].rearrange("p (c h) -> p c h", h=HB)
                        nc.vector.tensor_tensor(pv, xview(r0, di),
                                                kwview(r0, k), op=OP.mult)
                    if DVE_ADDS:
                        nc.vector.tensor_add(pd[:], pd[:], prod[:])
                    else:
                        nc.gpsimd.dma_start(pd[:], prod[:], accum_op=OP.add)
                pds.append(pd)
            pd_of[b] = ("mix", pds)

        def emit_c_mms(b):
            mode, pds = pd_of.pop(b)
            numt = nump.tile([W, C * HB], bf16, tag="numt")
            for t in range(3):
                acc = pcp.tile([W, CHUNK], f32, tag="pc_ps")
                if mode == "pe":
                    for n, (dj, prod) in enumerate(pds):
                        nc.tensor.matmul(
                            acc[:], lhsT=smat_sb[:, dj * W:(dj + 1) * W],
                            rhs=prod[:, t * CHUNK:(t + 1) * CHUNK],
                            start=(n == 0), stop=(n == 8))
                else:
                    for dj in range(3):
                        nc.tensor.matmul(
                            acc[:], lhsT=smat_sb[:, dj * W:(dj + 1) * W],
                            rhs=pds[dj][:, t * CHUNK:(t + 1) * CHUNK],
                            start=(dj == 0), stop=(dj == 2))
                nc.scalar.copy(numt[:, t * CHUNK:(t + 1) * CHUNK], acc[:])
            nc.sync.dma_start(
                y_d.ap()[:, b * C * HB:(b + 1) * C * HB], numt[:])

        # ---- band-pipelined emission: products lead their MMs by a group
        # so the accum-DMA chains have time to drain.
        groups = [(0, 1), (1, 3), (3, 5), (5, 7), (7, 8)]
        emitted = 0
        mm_done = 0
        for bs, be in groups:
            need = min(NCHUNK,
                       ((be * HB - 1) * RS + W + CHUNK - 1) // CHUNK)
            while emitted < need:
                n = min(2, need - emitted)
                emit_chunk_pair(emitted, n)
                emitted += n
            emit_b(bs * HB, (be - bs) * HB)
            for b in range(bs, be):
                emit_c_products(b)
            while mm_done < max(0, bs):
                emit_c_mms(mm_done)
                mm_done += 1
        while mm_done < NB:
            emit_c_mms(mm_done)
            mm_done += 1
        assert emitted == NCHUNK, emitted

    nc.compile()
    return nc


def _get_nc():
    if "nc" not in _CACHE:
        _CACHE["nc"] = _build()
    return _CACHE["nc"]


def kernel(x, dw_w, dw_b, pw1_w, pw1_b, pw2_w, pw2_b):
    from concourse.bass_utils import run_bass_kernel_spmd

    x = np.asarray(x, np.float32)
    dw_w = np.asarray(dw_w, np.float32)
    dw_b = np.asarray(dw_b, np.float32)
    pw1_w = np.asarray(pw1_w, np.float32)
    pw1_b = np.asarray(pw1_b, np.float32)
    pw2_w = np.asarray(pw2_w, np.float32)
    pw2_b = np.asarray(pw2_b, np.float32)

    bf = ml_dtypes.bfloat16
    f8 = ml_dtypes.float8_e4m3fn
    B = x.shape[0]
    # fused weights: wk[c, k, o] = pw1_w[o, c] * dw_w[c, 0, k//3, k%3],
    # pre-scaled 2^WS for fp8; packed into 5 DoubleRow pairs [c, p, j, o]
    wk = np.empty((C, K, CO), np.float32)
    for k in range(K):
        wk[:, k, :] = pw1_w.T * dw_w[:, 0, k // 3, k % 3][:, None]
    wk *= 2.0 ** WS
    wk5 = np.zeros((C, 5, 2, CO), np.float32)
    for p, (ka, kb) in enumerate(PAIRS):
        wk5[:, p, 0, :] = wk[:, ka, :]
        if kb is not None:
            wk5[:, p, 1, :] = wk[:, kb, :]
    wk5 = wk5.reshape(C, 5 * 2 * CO).astype(f8)
    pw2t = np.concatenate(
        [pw2_w.T, pw2_b[None, :]], axis=0).astype(bf)
    bh2 = (pw1_w @ dw_b + pw1_b).reshape(CO, 1).astype(np.float32)
    al = np.full((CO, 1), 0.2, np.float32)
    smat = np.concatenate(
        [np.eye(W, k=1), np.eye(W), np.eye(W, k=-1)],
        axis=1).astype(np.float32).astype(bf)
    ones = np.ones((1, NQ), np.float32).astype(bf)

    x_flat = np.zeros((B, C, 132, RS), f8)
    x_flat[:, :, 1:1 + H, 1:1 + W] = x.astype(f8)
    x_flat = x_flat.reshape(B, C, XBF)
    xT = x.transpose(0, 3, 1, 2).astype(bf)     # [B, w, c, h]
    xt2c = np.zeros((B, W, C, RS), bf)
    xt2c[:, :, :, 1:1 + H] = xT
    xt2c = xt2c.reshape(B, W, C * RS)

    nc = _get_nc()
    in_maps = [
        {"x_flat": x_flat[b], "xt2c": xt2c[b], "wk5": wk5, "pw2t": pw2t,
         "bh2": bh2, "al": al, "smat": smat, "ones": ones}
        for b in range(B)
    ]
    res = run_bass_kernel_spmd(nc, in_maps, core_ids=list(range(8)),
                               **_CACHE.get("run_kwargs", {}))
    _CACHE["last_result"] = res
    out = np.empty((B, C, H, W), np.float32)
    for b in range(B):
        yb = res.results[b]["y"].astype(np.float32)     # [w, (b, c, h)]
        out[b] = yb.reshape(W, NB, C, HB).transpose(2, 1, 3, 0).reshape(C, H, W)
    return out


# revision 8
# speedup vs baseline: 1.0014x; 1.0014x over previous
"""AdaptiveLowPassFilter Trainium2 kernel v4 — 8 NeuronCores, batch-parallel.

Measured-on-HW design points (vs v3's 281us / v2's 138us):
  - Products use tensor_tensor (2x_1p = 958ns per [128,1536] band-tap on
    HW). scalar_tensor_tensor never exceeds 1x on real TRN2 despite the
    cost model's 4x_2p table.
  - Intra-dj-column adds ride SWDGE accumulating DMAs (accum_op=add):
    ~0.94us of Pool-engine trigger each, transfer on idle DMA engines —
    the DVE keeps only the 72 products.
  - Phase A: fp8e4 DoubleRow pairs (2 taps/stream, j-stride must be
    EVEN on HW -> taps paired by delta parity). Prelu evac with an AP
    alpha (Lrelu's immediate alpha is silently ignored by HW) and
    scale=2^-8 compensating the fp8 weight pre-scale; 2-chunk paired
    evacs ([48,1024] across 2 psum banks) halve ACT instruction count.
  - Phase C dj-shift via shifted-identity matmuls (eye(k=+-1)) with
    kwA/kwB partition-shifted kw copies (HWDGE queue; SWDGE wedges the
    device on 3D strided copies).
"""
import sys
sys.path.insert(0, "/opt/trn_rl_repo")

import numpy as np
import ml_dtypes
from contextlib import ExitStack

C, CO, H, W, K = 96, 48, 128, 128, 9
RS = 130            # padded row stride in flat pixel space (and h stride)
PIX0 = 131          # flat offset of pixel (0, 0)
XBF = 17160         # x_flat free size (132 rows x 130)
NQ = 16896          # 33 chunks x 512 of h2 pixel space
NCHUNK = 33
CHUNK = 512
HB = 16             # band height (rows)
NB = H // HB        # 8 bands
WS = 8              # phase A weight pre-scale exponent (2^WS)

# tap deltas in flat pixel space, k = 3*di + dj
DELTA = [(di - 1) * RS + (dj - 1) for di in range(3) for dj in range(3)]
# DoubleRow tap pairs: j-stride (delta difference) must be even on HW
PAIRS = [(0, 2), (3, 5), (6, 8), (1, 4), (7, None)]
DUMMY_DELTA = 132   # valid x_flat memory, zero weights; 132-130=2 even

_CACHE = {}


def _build():
    import os
    import concourse.bass as bass
    import concourse.bacc as bacc
    import concourse.tile as tile
    import concourse.mybir as mybir

    DVE_ADDS = os.environ.get("V4_SWDGE_ADDS") != "1"
    POOL_TAPS = tuple(
        int(t) for t in os.environ.get("V4_POOL_TAPS", "").split(",") if t)
    PE_BANDS = tuple(
        int(b) for b in os.environ.get("V4_PE_BANDS", "").split(",") if b)
    POOL_REDUCE = os.environ.get("V4_POOL_REDUCE", "0") == "1"

    dt = mybir.dt
    f32, bf16, fp8 = dt.float32, dt.bfloat16, dt.float8e4
    AF = mybir.ActivationFunctionType
    OP = mybir.AluOpType
    PM = mybir.MatmulPerfMode

    nc = bacc.Bacc("TRN2", target_bir_lowering=False, debug=False)
    xf_d = nc.dram_tensor("x_flat", (C, XBF), fp8, kind="ExternalInput")
    xt_d = nc.dram_tensor("xt2c", (W, C * RS), bf16, kind="ExternalInput")
    wk5_d = nc.dram_tensor("wk5", (C, 5 * 2 * CO), fp8, kind="ExternalInput")
    pw2t_d = nc.dram_tensor("pw2t", (CO + 1, K), bf16, kind="ExternalInput")
    bh2_d = nc.dram_tensor("bh2", (CO, 1), f32, kind="ExternalInput")
    al_d = nc.dram_tensor("al", (CO, 1), f32, kind="ExternalInput")
    smat_d = nc.dram_tensor("smat", (W, 3 * W), bf16, kind="ExternalInput")
    ones_d = nc.dram_tensor("ones", (1, NQ), bf16, kind="ExternalInput")
    y_d = nc.dram_tensor("y", (W, NB * C * HB), bf16, kind="ExternalOutput")

    with ExitStack() as ctx:
        tc = ctx.enter_context(tile.TileContext(nc))
        st = ctx.enter_context(tc.tile_pool(name="st", bufs=1))
        prp = ctx.enter_context(tc.tile_pool(name="prp", bufs=12))
        pdp = ctx.enter_context(tc.tile_pool(name="pdp", bufs=6))
        nump = ctx.enter_context(tc.tile_pool(name="nump", bufs=3))
        h2p = ctx.enter_context(tc.tile_pool(name="h2p", bufs=2, space="PSUM"))
        ltp = ctx.enter_context(tc.tile_pool(name="ltp", bufs=2, space="PSUM"))
        pcp = ctx.enter_context(tc.tile_pool(name="pcp", bufs=2, space="PSUM"))

        x_flat = st.tile([C, XBF], fp8, tag="x_flat")
        xt2c = st.tile([W, C * RS], bf16, tag="xt2c")
        h2a = st.tile([W, NQ], bf16, tag="h2a")
        e_t = st.tile([W, K * H], bf16, tag="e_t")
        kw2 = st.tile([W, K * H], bf16, tag="kw2")
        kwA = st.tile([W, K * H], bf16, tag="kwA")
        kwB = st.tile([W, K * H], bf16, tag="kwB")
        den = st.tile([W, H], f32, tag="den")
        recip = st.tile([W, H], f32, tag="recip")
        wk5_sb = st.tile([C, 5 * 2 * CO], fp8, tag="wk5_sb")
        pw2t_sb = st.tile([CO + 1, K], bf16, tag="pw2t_sb")
        bh2_sb = st.tile([CO, 1], f32, tag="bh2_sb")
        al_sb = st.tile([CO, 1], f32, tag="al_sb")
        smat_sb = st.tile([W, 3 * W], bf16, tag="smat_sb")

        # ---- params (small, first; smat leads so PE warm-up starts early)
        nc.scalar.dma_start(smat_sb[:], smat_d.ap())
        nc.scalar.dma_start(wk5_sb[:], wk5_d.ap())
        nc.scalar.dma_start(pw2t_sb[:], pw2t_d.ap())
        nc.scalar.dma_start(bh2_sb[:], bh2_d.ap())
        nc.scalar.dma_start(al_sb[:], al_d.ap())
        nc.scalar.dma_start(h2a[CO:CO + 1, :], ones_d.ap())

        # ---- bulk inputs; x_flat leads on the SP queue, xt2c h-slabs on
        # the scalar queue (c-major layout -> strided pieces).
        XCUT = [0, 2145, 4290, 8580, 12870, XBF]
        for j in range(2):
            nc.sync.dma_start(
                x_flat[:, XCUT[j]:XCUT[j + 1]],
                xf_d.ap()[:, XCUT[j]:XCUT[j + 1]])
        HCUT = [0, 34, 66, 98, RS]
        xt_v = xt2c[:].rearrange("p (c h) -> p c h", h=RS)
        xtd_v = xt_d.ap().rearrange("p (c h) -> p c h", h=RS)
        nc.scalar.dma_start(xt_v[:, :, HCUT[0]:HCUT[1]],
                            xtd_v[:, :, HCUT[0]:HCUT[1]])
        for j in range(2, 5):
            nc.sync.dma_start(
                x_flat[:, XCUT[j]:XCUT[j + 1]],
                xf_d.ap()[:, XCUT[j]:XCUT[j + 1]])
        for j in range(1, 4):
            nc.scalar.dma_start(xt_v[:, :, HCUT[j]:HCUT[j + 1]],
                                xtd_v[:, :, HCUT[j]:HCUT[j + 1]])

        # ---- PE warm-up: real matmuls so HAM reaches 2.4 GHz
        wup = pcp.tile([W, CHUNK], f32, tag="pc_ps")
        for _ in range(12):
            nc.tensor.matmul(wup[:, 0:W], lhsT=smat_sb[:, W:2 * W],
                             rhs=smat_sb[:, W:2 * W], start=True, stop=True)

        # ---- phase A: fused dw*pw1 conv, fp8 DoubleRow tap pairs;
        # chunks processed in PAIRS sharing a 2-bank psum tile so one
        # Prelu evac covers 1024 columns.
        lhs5 = wk5_sb[:].rearrange("c (p j o) -> c p j o", p=5, j=2)
        xf_ap = x_flat[:]

        def emit_chunk_pair(i0, n):
            ps = h2p.tile([CO, 2 * CHUNK], f32, tag="h2ps")
            for t in range(n):
                q0 = PIX0 + CHUNK * (i0 + t)
                blk = ps[:, t * CHUNK:(t + 1) * CHUNK]
                for p, (ka, kb) in enumerate(PAIRS):
                    da = DELTA[ka]
                    db = DELTA[kb] if kb is not None else DUMMY_DELTA
                    rhs = bass.AP(xf_ap.tensor, xf_ap.offset + q0 + da,
                                  [[XBF, C], [db - da, 2], [1, CHUNK]])
                    nc.tensor.matmul(blk, lhsT=lhs5[:, p], rhs=rhs,
                                     start=(p == 0), stop=(p == 4),
                                     perf_mode=PM.DoubleRow)
            nc.scalar.activation(
                h2a[0:CO, CHUNK * i0:CHUNK * (i0 + n)], ps[:, 0:n * CHUNK],
                AF.Prelu, bias=bh2_sb[:], scale=2.0 ** -WS, alpha=al_sb[:],
            )

        # ---- phase B + softmax weights for a run of rows; e_t/kw2 are
        # stored [w, (k, h)] (h contiguous). kwA/kwB are the +/-1-partition
        # shifted copies of kw2.
        ev = e_t[:].rearrange("p (k h) -> p k h", h=H)
        kv2 = kw2[:].rearrange("p (k h) -> p k h", h=H)
        kvA = kwA[:].rearrange("p (k h) -> p k h", h=H)
        kvB = kwB[:].rearrange("p (k h) -> p k h", h=H)

        def emit_b(r0, rn):
            lt = ltp.tile([W, 2 * HB * K], f32, tag="lt")
            for r in range(rn):
                h = r0 + r
                nc.tensor.matmul(
                    lt[:, r * K:(r + 1) * K],
                    lhsT=h2a[0:CO + 1, h * RS:h * RS + W],
                    rhs=pw2t_sb[:], start=True, stop=True)
            ltv = lt[:, 0:rn * K].rearrange("p (h k) -> p k h", k=K)
            eb = ev[:, :, r0:r0 + rn]
            nc.scalar.activation(eb, ltv, AF.Exp)
            db = den[:, r0:r0 + rn]
            red_eng = nc.gpsimd if POOL_REDUCE else nc.vector
            red_eng.tensor_reduce(
                db, eb.transpose([0, 2, 1]),
                axis=mybir.AxisListType.X, op=OP.add)
            rb = recip[:, r0:r0 + rn]
            nc.vector.reciprocal(rb, db)
            nc.vector.tensor_tensor(
                kv2[:, :, r0:r0 + rn], eb,
                rb.unsqueeze(1).broadcast_to([W, K, rn]),
                op=OP.mult,
            )
            # kwA[v] = kw2[v+1] (needs dj=0 taps k=0,3,6): lhsT=eye(k=-1);
            # kwB[v] = kw2[v-1] (dj=2 taps k=2,5,8): lhsT=eye(k=+1).
            # psum boundary rows come out zero, exactly what the conv pads
            # need.
            for tgt, blk, k0 in ((kvA, 2, 0), (kvB, 0, 2)):
                kps = pcp.tile([W, CHUNK], f32, tag="pc_ps")
                rhs = bass.AP(kv2.tensor, kv2.offset + k0 * H + r0,
                              [[K * H, W], [3 * H, 3], [1, rn]])
                nc.tensor.matmul(kps[:, 0:3 * rn],
                                 lhsT=smat_sb[:, blk * W:(blk + 1) * W],
                                 rhs=rhs, start=True, stop=True)
                out = bass.AP(tgt.tensor, tgt.offset + k0 * H + r0,
                              [[K * H, W], [3 * H, 3], [1, rn]])
                nc.scalar.copy(out, kps[:, 0:3 * rn])

        KWT = {0: kvA, 1: kv2, 2: kvB}
        xcv = xt2c[:].rearrange("p (c h) -> p c h", h=RS)

        def xview(r0, di):
            return xcv[:, :, r0 + di:r0 + di + HB]

        def kwview(r0, k):
            return (KWT[k % 3][:, k, r0:r0 + HB]
                    .unsqueeze(1).broadcast_to([W, C, HB]))

        # ---- phase C: per-pixel 3x3 apply. Products on DVE (+ a few on
        # Pool); the 2 intra-dj adds per column ride SWDGE accum-DMAs; the
        # dj w-shift via shifted-identity matmuls on PE; evac on ACT.
        pd_of = {}

        def emit_c_products(b):
            r0 = b * HB
            # pool-tap products first so the Pool engine leads the chain
            pool_prods = {}
            for di in range(3):
                for dj in range(3):
                    k = 3 * di + dj
                    if k in POOL_TAPS:
                        prod = prp.tile([W, C * HB], bf16, tag="prod")
                        pv = prod[:].rearrange("p (c h) -> p c h", h=HB)
                        nc.gpsimd.tensor_tensor(
                            pv, xview(r0, di), kwview(r0, k), op=OP.mult)
                        pool_prods[k] = prod
            if b in PE_BANDS:
                prods = []
                for di in range(3):
                    for dj in range(3):
                        k = 3 * di + dj
                        if k in pool_prods:
                            prods.append((dj, pool_prods[k]))
                            continue
                        prod = prp.tile([W, C * HB], bf16, tag="prod")
                        pv = prod[:].rearrange("p (c h) -> p c h", h=HB)
                        nc.vector.tensor_tensor(
                            pv, xview(r0, di), kwview(r0, k), op=OP.mult)
                        prods.append((dj, prod))
                pd_of[b] = ("pe", prods)
                return
            pds = []
            for dj in range(3):
                pd = pdp.tile([W, C * HB], bf16, tag="pd")
                pdv = pd[:].rearrange("p (c h) -> p c h", h=HB)
                nc.vector.tensor_tensor(pdv, xview(r0, 0), kwview(r0, dj),
                                        op=OP.mult)
                for di in (1, 2):
                    k = 3 * di + dj
                    if k in pool_prods:
                        prod = pool_prods[k]
                    else:
                        prod = prp.tile([W, C * HB], bf16, tag="prod")
                        pv = prod[:].rearrange("p (c h) -> p c h", h=HB)
                        nc.vector.tensor_tensor(pv, xview(r0, di),
                                                kwview(r0, k), op=OP.mult)
                    if DVE_ADDS:
                        nc.vector.tensor_add(pd[:], pd[:], prod[:])
                    else:
                        nc.gpsimd.dma_start(pd[:], prod[:], accum_op=OP.add)
                pds.append(pd)
            pd_of[b] = ("mix", pds)

        def emit_c_mms(b):
            mode, pds = pd_of.pop(b)
            numt = nump.tile([W, C * HB], bf16, tag="numt")
            for t in range(3):
                acc = pcp.tile([W, CHUNK], f32, tag="pc_ps")
                if mode == "pe":
                    for n, (dj, prod) in enumerate(pds):
                        nc.tensor.matmul(
                            acc[:], lhsT=smat_sb[:, dj * W:(dj + 1) * W],
                            rhs=prod[:, t * CHUNK:(t + 1) * CHUNK],
                            start=(n == 0), stop=(n == 8))
                else:
                    for dj in range(3):
                        nc.tensor.matmul(
                            acc[:], lhsT=smat_sb[:, dj * W:(dj + 1) * W],
                            rhs=pds[dj][:, t * CHUNK:(t + 1) * CHUNK],
                            start=(dj == 0), stop=(dj == 2))
                nc.scalar.copy(numt[:, t * CHUNK:(t + 1) * CHUNK], acc[:])
            nc.sync.dma_start(
                y_d.ap()[:, b * C * HB:(b + 1) * C * HB], numt[:])

        # ---- band-pipelined emission: products lead their MMs by a group
        # so the accum-DMA chains have time to drain.
        emitted = 0
        mm_done = 0
        for b in range(NB):
            need = min(NCHUNK,
                       (((b + 1) * HB - 1) * RS + W + CHUNK - 1) // CHUNK)
            while emitted < need:
                n = min(2, need - emitted)
                emit_chunk_pair(emitted, n)
                emitted += n
            emit_b(b * HB, HB)
            emit_c_products(b)
            if b >= 1:
                emit_c_mms(b - 1)
                mm_done = b
        emit_c_mms(NB - 1)
        assert emitted == NCHUNK, emitted

    nc.compile()
    return nc


def _get_nc():
    if "nc" not in _CACHE:
        _CACHE["nc"] = _build()
    return _CACHE["nc"]


def kernel(x, dw_w, dw_b, pw1_w, pw1_b, pw2_w, pw2_b):
    from concourse.bass_utils import run_bass_kernel_spmd

    x = np.asarray(x, np.float32)
    dw_w = np.asarray(dw_w, np.float32)
    dw_b = np.asarray(dw_b, np.float32)
    pw1_w = np.asarray(pw1_w, np.float32)
    pw1_b = np.asarray(pw1_b, np.float32)
    pw2_w = np.asarray(pw2_w, np.float32)
    pw2_b = np.asarray(pw2_b, np.float32)

    bf = ml_dtypes.bfloat16
    f8 = ml_dtypes.float8_e4m3fn
    B = x.shape[0]
    # fused weights: wk[c, k, o] = pw1_w[o, c] * dw_w[c, 0, k//3, k%3],
    # pre-scaled 2^WS for fp8; packed into 5 DoubleRow pairs [c, p, j, o]
    wk = np.empty((C, K, CO), np.float32)
    for k in range(K):
        wk[:, k, :] = pw1_w.T * dw_w[:, 0, k // 3, k % 3][:, None]
    wk *= 2.0 ** WS
    wk5 = np.zeros((C, 5, 2, CO), np.float32)
    for p, (ka, kb) in enumerate(PAIRS):
        wk5[:, p, 0, :] = wk[:, ka, :]
        if kb is not None:
            wk5[:, p, 1, :] = wk[:, kb, :]
    wk5 = wk5.reshape(C, 5 * 2 * CO).astype(f8)
    pw2t = np.concatenate(
        [pw2_w.T, pw2_b[None, :]], axis=0).astype(bf)
    bh2 = (pw1_w @ dw_b + pw1_b).reshape(CO, 1).astype(np.float32)
    al = np.full((CO, 1), 0.2, np.float32)
    smat = np.concatenate(
        [np.eye(W, k=1), np.eye(W), np.eye(W, k=-1)],
        axis=1).astype(np.float32).astype(bf)
    ones = np.ones((1, NQ), np.float32).astype(bf)

    x_flat = np.zeros((B, C, 132, RS), f8)
    x_flat[:, :, 1:1 + H, 1:1 + W] = x.astype(f8)
    x_flat = x_flat.reshape(B, C, XBF)
    xT = x.transpose(0, 3, 1, 2).astype(bf)     # [B, w, c, h]
    xt2c = np.zeros((B, W, C, RS), bf)
    xt2c[:, :, :, 1:1 + H] = xT
    xt2c = xt2c.reshape(B, W, C * RS)

    nc = _get_nc()
    in_maps = [
        {"x_flat": x_flat[b], "xt2c": xt2c[b], "wk5": wk5, "pw2t": pw2t,
         "bh2": bh2, "al": al, "smat": smat, "ones": ones}
        for b in range(B)
    ]
    res = run_bass_kernel_spmd(nc, in_maps, core_ids=list(range(8)),
                               **_CACHE.get("run_kwargs", {}))
    _CACHE["last_result"] = res
    out = np.empty((B, C, H, W), np.float32)
    for b in range(B):
        yb = res.results[b]["y"].astype(np.float32)     # [w, (b, c, h)]
        out[b] = yb.reshape(W, NB, C, HB).transpose(2, 1, 3, 0).reshape(C, H, W)
    return out


# revision 9
# speedup vs baseline: 1.1170x; 1.1155x over previous
"""AdaptiveLowPassFilter Trainium2 kernel v4 — 8 NeuronCores, batch-parallel.

Measured-on-HW design points (vs v3's 281us / v2's 138us):
  - Products use tensor_tensor (2x_1p = 958ns per [128,1536] band-tap on
    HW). scalar_tensor_tensor never exceeds 1x on real TRN2 despite the
    cost model's 4x_2p table.
  - Intra-dj-column adds ride SWDGE accumulating DMAs (accum_op=add):
    ~0.94us of Pool-engine trigger each, transfer on idle DMA engines —
    the DVE keeps only the 72 products.
  - Phase A: fp8e4 DoubleRow pairs (2 taps/stream, j-stride must be
    EVEN on HW -> taps paired by delta parity). Prelu evac with an AP
    alpha (Lrelu's immediate alpha is silently ignored by HW) and
    scale=2^-8 compensating the fp8 weight pre-scale; 2-chunk paired
    evacs ([48,1024] across 2 psum banks) halve ACT instruction count.
  - Phase C dj-shift via shifted-identity matmuls (eye(k=+-1)) with
    kwA/kwB partition-shifted kw copies (HWDGE queue; SWDGE wedges the
    device on 3D strided copies).
"""
import sys
sys.path.insert(0, "/opt/trn_rl_repo")

import numpy as np
import ml_dtypes
from contextlib import ExitStack

C, CO, H, W, K = 96, 48, 128, 128, 9
RS = 130            # padded row stride in flat pixel space (and h stride)
SLH = 18            # per-band slab height (HB + 2 halo rows)
PIX0 = 131          # flat offset of pixel (0, 0)
XBF = 17160         # x_flat free size (132 rows x 130)
NQ = 16896          # 33 chunks x 512 of h2 pixel space
NCHUNK = 33
CHUNK = 512
HB = 16             # band height (rows)
NB = H // HB        # 8 bands
WS = 8              # phase A weight pre-scale exponent (2^WS)

# tap deltas in flat pixel space, k = 3*di + dj
DELTA = [(di - 1) * RS + (dj - 1) for di in range(3) for dj in range(3)]
# DoubleRow tap pairs: j-stride (delta difference) must be even on HW
PAIRS = [(0, 2), (3, 5), (6, 8), (1, 4), (7, None)]
DUMMY_DELTA = 132   # valid x_flat memory, zero weights; 132-130=2 even

_CACHE = {}


def _build():
    import os
    import concourse.bass as bass
    import concourse.bacc as bacc
    import concourse.tile as tile
    import concourse.mybir as mybir

    DVE_ADDS = os.environ.get("V4_SWDGE_ADDS") != "1"
    POOL_TAPS = tuple(
        int(t) for t in os.environ.get("V4_POOL_TAPS", "").split(",") if t)
    PE_BANDS = tuple(
        int(b) for b in os.environ.get("V4_PE_BANDS", "").split(",") if b)
    POOL_REDUCE = os.environ.get("V4_POOL_REDUCE", "0") == "1"

    dt = mybir.dt
    f32, bf16, fp8 = dt.float32, dt.bfloat16, dt.float8e4
    AF = mybir.ActivationFunctionType
    OP = mybir.AluOpType
    PM = mybir.MatmulPerfMode

    nc = bacc.Bacc("TRN2", target_bir_lowering=False, debug=False)
    xf_d = nc.dram_tensor("x_flat", (C, XBF), fp8, kind="ExternalInput")
    xt_d = nc.dram_tensor("xsl", (W, NB * C * SLH), bf16, kind="ExternalInput")
    wk5_d = nc.dram_tensor("wk5", (C, 5 * 2 * CO), fp8, kind="ExternalInput")
    pw2t_d = nc.dram_tensor("pw2t", (CO + 1, K), bf16, kind="ExternalInput")
    bh2_d = nc.dram_tensor("bh2", (CO, 1), f32, kind="ExternalInput")
    al_d = nc.dram_tensor("al", (CO, 1), f32, kind="ExternalInput")
    smat_d = nc.dram_tensor("smat", (W, 3 * W), bf16, kind="ExternalInput")
    ones_d = nc.dram_tensor("ones", (1, NQ), bf16, kind="ExternalInput")
    y_d = nc.dram_tensor("y", (W, NB * C * HB), bf16, kind="ExternalOutput")

    with ExitStack() as ctx:
        tc = ctx.enter_context(tile.TileContext(nc))
        st = ctx.enter_context(tc.tile_pool(name="st", bufs=1))
        prp = ctx.enter_context(tc.tile_pool(name="prp", bufs=12))
        pdp = ctx.enter_context(tc.tile_pool(name="pdp", bufs=6))
        nump = ctx.enter_context(tc.tile_pool(name="nump", bufs=3))
        h2p = ctx.enter_context(tc.tile_pool(name="h2p", bufs=2, space="PSUM"))
        ltp = ctx.enter_context(tc.tile_pool(name="ltp", bufs=2, space="PSUM"))
        pcp = ctx.enter_context(tc.tile_pool(name="pcp", bufs=2, space="PSUM"))

        x_flat = st.tile([C, XBF], fp8, tag="x_flat")
        xsl = st.tile([W, NB * C * SLH], bf16, tag="xsl")
        h2a = st.tile([W, NQ], bf16, tag="h2a")
        e_t = st.tile([W, K * H], bf16, tag="e_t")
        kw2 = st.tile([W, K * H], bf16, tag="kw2")
        kwA = st.tile([W, K * H], bf16, tag="kwA")
        kwB = st.tile([W, K * H], bf16, tag="kwB")
        den = st.tile([W, H], f32, tag="den")
        recip = st.tile([W, H], f32, tag="recip")
        wk5_sb = st.tile([C, 5 * 2 * CO], fp8, tag="wk5_sb")
        pw2t_sb = st.tile([CO + 1, K], bf16, tag="pw2t_sb")
        bh2_sb = st.tile([CO, 1], f32, tag="bh2_sb")
        al_sb = st.tile([CO, 1], f32, tag="al_sb")
        smat_sb = st.tile([W, 3 * W], bf16, tag="smat_sb")

        # ---- sync (SP) queue: ones + phase-A params, x_flat pieces, smat.
        # scalar (ACT) queue: the 8 contiguous per-band x slabs (128
        # descriptors each; the old strided h-slabs cost ~49k descriptors
        # and ground all 16 DMA engines for ~50us).
        nc.sync.dma_start(h2a[CO:CO + 1, :], ones_d.ap())
        nc.sync.dma_start(wk5_sb[:], wk5_d.ap())
        nc.sync.dma_start(pw2t_sb[:], pw2t_d.ap())
        nc.sync.dma_start(bh2_sb[:], bh2_d.ap())
        nc.sync.dma_start(al_sb[:], al_d.ap())
        XCUT = [0, 2145, 4290, 8580, 12870, XBF]
        for j in range(2):
            nc.sync.dma_start(
                x_flat[:, XCUT[j]:XCUT[j + 1]],
                xf_d.ap()[:, XCUT[j]:XCUT[j + 1]])
        nc.sync.dma_start(smat_sb[:], smat_d.ap())
        for j in range(2, 5):
            nc.sync.dma_start(
                x_flat[:, XCUT[j]:XCUT[j + 1]],
                xf_d.ap()[:, XCUT[j]:XCUT[j + 1]])
        for b in range(NB):
            sl = slice(b * C * SLH, (b + 1) * C * SLH)
            nc.scalar.dma_start(xsl[:, sl], xt_d.ap()[:, sl])

        # ---- phase A: fused dw*pw1 conv, fp8 DoubleRow tap pairs;
        # chunks processed in PAIRS sharing a 2-bank psum tile so one
        # Prelu evac covers 1024 columns.
        lhs5 = wk5_sb[:].rearrange("c (p j o) -> c p j o", p=5, j=2)
        xf_ap = x_flat[:]

        def emit_chunk_pair(i0, n):
            ps = h2p.tile([CO, 2 * CHUNK], f32, tag="h2ps")
            for t in range(n):
                q0 = PIX0 + CHUNK * (i0 + t)
                blk = ps[:, t * CHUNK:(t + 1) * CHUNK]
                for p, (ka, kb) in enumerate(PAIRS):
                    da = DELTA[ka]
                    db = DELTA[kb] if kb is not None else DUMMY_DELTA
                    rhs = bass.AP(xf_ap.tensor, xf_ap.offset + q0 + da,
                                  [[XBF, C], [db - da, 2], [1, CHUNK]])
                    nc.tensor.matmul(blk, lhsT=lhs5[:, p], rhs=rhs,
                                     start=(p == 0), stop=(p == 4),
                                     perf_mode=PM.DoubleRow)
            nc.scalar.activation(
                h2a[0:CO, CHUNK * i0:CHUNK * (i0 + n)], ps[:, 0:n * CHUNK],
                AF.Prelu, bias=bh2_sb[:], scale=2.0 ** -WS, alpha=al_sb[:],
            )

        # ---- phase B + softmax weights for a run of rows; e_t/kw2 are
        # stored [w, (k, h)] (h contiguous). kwA/kwB are the +/-1-partition
        # shifted copies of kw2.
        ev = e_t[:].rearrange("p (k h) -> p k h", h=H)
        kv2 = kw2[:].rearrange("p (k h) -> p k h", h=H)
        kvA = kwA[:].rearrange("p (k h) -> p k h", h=H)
        kvB = kwB[:].rearrange("p (k h) -> p k h", h=H)

        def emit_b(r0, rn):
            lt = ltp.tile([W, 2 * HB * K], f32, tag="lt")
            for r in range(rn):
                h = r0 + r
                nc.tensor.matmul(
                    lt[:, r * K:(r + 1) * K],
                    lhsT=h2a[0:CO + 1, h * RS:h * RS + W],
                    rhs=pw2t_sb[:], start=True, stop=True)
            ltv = lt[:, 0:rn * K].rearrange("p (h k) -> p k h", k=K)
            eb = ev[:, :, r0:r0 + rn]
            nc.scalar.activation(eb, ltv, AF.Exp)
            db = den[:, r0:r0 + rn]
            red_eng = nc.gpsimd if POOL_REDUCE else nc.vector
            red_eng.tensor_reduce(
                db, eb.transpose([0, 2, 1]),
                axis=mybir.AxisListType.X, op=OP.add)
            rb = recip[:, r0:r0 + rn]
            nc.vector.reciprocal(rb, db)
            nc.vector.tensor_tensor(
                kv2[:, :, r0:r0 + rn], eb,
                rb.unsqueeze(1).broadcast_to([W, K, rn]),
                op=OP.mult,
            )
            # kwA[v] = kw2[v+1] (needs dj=0 taps k=0,3,6): lhsT=eye(k=-1);
            # kwB[v] = kw2[v-1] (dj=2 taps k=2,5,8): lhsT=eye(k=+1).
            # psum boundary rows come out zero, exactly what the conv pads
            # need.
            for tgt, blk, k0 in ((kvA, 2, 0), (kvB, 0, 2)):
                kps = pcp.tile([W, CHUNK], f32, tag="pc_ps")
                rhs = bass.AP(kv2.tensor, kv2.offset + k0 * H + r0,
                              [[K * H, W], [3 * H, 3], [1, rn]])
                nc.tensor.matmul(kps[:, 0:3 * rn],
                                 lhsT=smat_sb[:, blk * W:(blk + 1) * W],
                                 rhs=rhs, start=True, stop=True)
                out = bass.AP(tgt.tensor, tgt.offset + k0 * H + r0,
                              [[K * H, W], [3 * H, 3], [1, rn]])
                nc.scalar.copy(out, kps[:, 0:3 * rn])

        KWT = {0: kvA, 1: kv2, 2: kvB}

        def xview(r0, di):
            b = r0 // HB
            blk = xsl[:, b * C * SLH:(b + 1) * C * SLH]
            return blk.rearrange("p (c h) -> p c h", h=SLH)[:, :, di:di + HB]

        def kwview(r0, k):
            return (KWT[k % 3][:, k, r0:r0 + HB]
                    .unsqueeze(1).broadcast_to([W, C, HB]))

        # ---- phase C: per-pixel 3x3 apply. Products on DVE (+ a few on
        # Pool); the 2 intra-dj adds per column ride SWDGE accum-DMAs; the
        # dj w-shift via shifted-identity matmuls on PE; evac on ACT.
        pd_of = {}

        def emit_c_products(b):
            r0 = b * HB
            # pool-tap products first so the Pool engine leads the chain
            pool_prods = {}
            for di in range(3):
                for dj in range(3):
                    k = 3 * di + dj
                    if k in POOL_TAPS:
                        prod = prp.tile([W, C * HB], bf16, tag="prod")
                        pv = prod[:].rearrange("p (c h) -> p c h", h=HB)
                        nc.gpsimd.tensor_tensor(
                            pv, xview(r0, di), kwview(r0, k), op=OP.mult)
                        pool_prods[k] = prod
            if b in PE_BANDS:
                prods = []
                for di in range(3):
                    for dj in range(3):
                        k = 3 * di + dj
                        if k in pool_prods:
                            prods.append((dj, pool_prods[k]))
                            continue
                        prod = prp.tile([W, C * HB], bf16, tag="prod")
                        pv = prod[:].rearrange("p (c h) -> p c h", h=HB)
                        nc.vector.tensor_tensor(
                            pv, xview(r0, di), kwview(r0, k), op=OP.mult)
                        prods.append((dj, prod))
                pd_of[b] = ("pe", prods)
                return
            pds = []
            for dj in range(3):
                pd = pdp.tile([W, C * HB], bf16, tag="pd")
                pdv = pd[:].rearrange("p (c h) -> p c h", h=HB)
                nc.vector.tensor_tensor(pdv, xview(r0, 0), kwview(r0, dj),
                                        op=OP.mult)
                for di in (1, 2):
                    k = 3 * di + dj
                    if k in pool_prods:
                        prod = pool_prods[k]
                    else:
                        prod = prp.tile([W, C * HB], bf16, tag="prod")
                        pv = prod[:].rearrange("p (c h) -> p c h", h=HB)
                        nc.vector.tensor_tensor(pv, xview(r0, di),
                                                kwview(r0, k), op=OP.mult)
                    if DVE_ADDS:
                        nc.vector.tensor_add(pd[:], pd[:], prod[:])
                    else:
                        nc.gpsimd.dma_start(pd[:], prod[:], accum_op=OP.add)
                pds.append(pd)
            pd_of[b] = ("mix", pds)

        def emit_c_mms(b):
            mode, pds = pd_of.pop(b)
            numt = nump.tile([W, C * HB], bf16, tag="numt")
            for t in range(3):
                acc = pcp.tile([W, CHUNK], f32, tag="pc_ps")
                if mode == "pe":
                    for n, (dj, prod) in enumerate(pds):
                        nc.tensor.matmul(
                            acc[:], lhsT=smat_sb[:, dj * W:(dj + 1) * W],
                            rhs=prod[:, t * CHUNK:(t + 1) * CHUNK],
                            start=(n == 0), stop=(n == 8))
                else:
                    for dj in range(3):
                        nc.tensor.matmul(
                            acc[:], lhsT=smat_sb[:, dj * W:(dj + 1) * W],
                            rhs=pds[dj][:, t * CHUNK:(t + 1) * CHUNK],
                            start=(dj == 0), stop=(dj == 2))
                nc.scalar.copy(numt[:, t * CHUNK:(t + 1) * CHUNK], acc[:])
            nc.sync.dma_start(
                y_d.ap()[:, b * C * HB:(b + 1) * C * HB], numt[:])

        # ---- band-pipelined emission: products lead their MMs by a group
        # so the accum-DMA chains have time to drain.
        emitted = 0
        mm_done = 0
        for b in range(NB):
            need = min(NCHUNK,
                       (((b + 1) * HB - 1) * RS + W + CHUNK - 1) // CHUNK)
            while emitted < need:
                n = min(2, need - emitted)
                emit_chunk_pair(emitted, n)
                emitted += n
            emit_b(b * HB, HB)
            emit_c_products(b)
            if b >= 1:
                emit_c_mms(b - 1)
                mm_done = b
        emit_c_mms(NB - 1)
        assert emitted == NCHUNK, emitted

    nc.compile()
    return nc


def _get_nc():
    if "nc" not in _CACHE:
        _CACHE["nc"] = _build()
    return _CACHE["nc"]


def kernel(x, dw_w, dw_b, pw1_w, pw1_b, pw2_w, pw2_b):
    from concourse.bass_utils import run_bass_kernel_spmd

    x = np.asarray(x, np.float32)
    dw_w = np.asarray(dw_w, np.float32)
    dw_b = np.asarray(dw_b, np.float32)
    pw1_w = np.asarray(pw1_w, np.float32)
    pw1_b = np.asarray(pw1_b, np.float32)
    pw2_w = np.asarray(pw2_w, np.float32)
    pw2_b = np.asarray(pw2_b, np.float32)

    bf = ml_dtypes.bfloat16
    f8 = ml_dtypes.float8_e4m3fn
    B = x.shape[0]
    # fused weights: wk[c, k, o] = pw1_w[o, c] * dw_w[c, 0, k//3, k%3],
    # pre-scaled 2^WS for fp8; packed into 5 DoubleRow pairs [c, p, j, o]
    wk = np.empty((C, K, CO), np.float32)
    for k in range(K):
        wk[:, k, :] = pw1_w.T * dw_w[:, 0, k // 3, k % 3][:, None]
    wk *= 2.0 ** WS
    wk5 = np.zeros((C, 5, 2, CO), np.float32)
    for p, (ka, kb) in enumerate(PAIRS):
        wk5[:, p, 0, :] = wk[:, ka, :]
        if kb is not None:
            wk5[:, p, 1, :] = wk[:, kb, :]
    wk5 = wk5.reshape(C, 5 * 2 * CO).astype(f8)
    pw2t = np.concatenate(
        [pw2_w.T, pw2_b[None, :]], axis=0).astype(bf)
    bh2 = (pw1_w @ dw_b + pw1_b).reshape(CO, 1).astype(np.float32)
    al = np.full((CO, 1), 0.2, np.float32)
    smat = np.concatenate(
        [np.eye(W, k=1), np.eye(W), np.eye(W, k=-1)],
        axis=1).astype(np.float32).astype(bf)
    ones = np.ones((1, NQ), np.float32).astype(bf)

    x_flat = np.zeros((B, C, 132, RS), f8)
    x_flat[:, :, 1:1 + H, 1:1 + W] = x.astype(f8)
    x_flat = x_flat.reshape(B, C, XBF)
    xT = x.transpose(0, 3, 1, 2).astype(bf)     # [B, w, c, h]
    xt2c = np.zeros((B, W, C, RS), bf)
    xt2c[:, :, :, 1:1 + H] = xT
    xsl = np.empty((B, W, NB, C, SLH), bf)
    for b in range(NB):
        xsl[:, :, b] = xt2c[:, :, :, HB * b:HB * b + SLH]
    xsl = xsl.reshape(B, W, NB * C * SLH)

    nc = _get_nc()
    in_maps = [
        {"x_flat": x_flat[b], "xsl": xsl[b], "wk5": wk5, "pw2t": pw2t,
         "bh2": bh2, "al": al, "smat": smat, "ones": ones}
        for b in range(B)
    ]
    res = run_bass_kernel_spmd(nc, in_maps, core_ids=list(range(8)),
                               **_CACHE.get("run_kwargs", {}))
    _CACHE["last_result"] = res
    out = np.empty((B, C, H, W), np.float32)
    for b in range(B):
        yb = res.results[b]["y"].astype(np.float32)     # [w, (b, c, h)]
        out[b] = yb.reshape(W, NB, C, HB).transpose(2, 1, 3, 0).reshape(C, H, W)
    return out
